# revision 4
# baseline (speedup 1.0000x reference)
"""TF-IDF document model (histogram_binning) on 8 TRN2 NeuronCores.

Algorithm (per core, 64 batch rows, data-parallel over batch):
  For each row b: tf histogram over vocab V=50257 computed as a radix
  one-hot matmul on the PE: vocab index v = hi*393 + lo with hi in
  [0,128), lo in [0,393). For each 128-token chunk of the row,
  A[s,hi] = (hi_s == hi), B[s,lo] = (lo_s == lo) (fp16 one-hots built
  by DVE tensor_scalar is_equal against an iota tile), and
  C[hi,lo] += A^T @ B accumulates the count matrix in PSUM.
  Then T = C * idf2 (idf reshaped [128,393]) with a fused per-partition
  row-sum; n_b = total sum via a ones-matmul; out_row = T / n_b.
Output written as [64, 50304] per core (vocab padded 50257->50304);
host slices/concats to (512, 50257).
"""
import numpy as np

import concourse.bacc as bacc
import concourse.mybir as mybir
from concourse import bass_utils
from concourse.tile import TileContext

B, S, V = 512, 1024, 50257
NC = 8
BL = B // NC          # 64 rows per core
HI, LO = 128, 393     # radix split: v = hi*LO + lo
VP = HI * LO          # 50304 padded vocab
CH = S // 128         # 8 token chunks per row
GROUP = 8             # rows per normalization group

_cache = {}


def _build():
    nc = bacc.Bacc(
        "TRN2",
        target_bir_lowering=False,
        debug=False,
        enable_asserts=False,
        num_devices=NC,
    )
    hif_t = nc.dram_tensor("hif", [128, BL * CH], mybir.dt.float32, kind="ExternalInput")
    lof_t = nc.dram_tensor("lof", [128, BL * CH], mybir.dt.float32, kind="ExternalInput")
    idf2_t = nc.dram_tensor("idf2", [HI, LO], mybir.dt.float32, kind="ExternalInput")
    iota_t = nc.dram_tensor("iota", [128, LO], mybir.dt.float16, kind="ExternalInput")
    onesc_t = nc.dram_tensor("onesc", [128, 1], mybir.dt.float32, kind="ExternalInput")
    onesr_t = nc.dram_tensor("onesr", [1, 128], mybir.dt.float32, kind="ExternalInput")
    out_t = nc.dram_tensor("out", [BL, VP], mybir.dt.float32, kind="ExternalOutput")
    ov = out_t.ap().rearrange("b (p f) -> b p f", p=HI)

    AF = mybir.ActivationFunctionType
    OP = mybir.AluOpType
    ncols = BL * CH

    with TileContext(nc) as tc:
        with (
            tc.tile_pool(name="const", bufs=1) as cpool,
            tc.tile_pool(name="work", bufs=4) as wpool,
            tc.tile_pool(name="tt", bufs=2 * GROUP + 2) as tpool,
            tc.tile_pool(name="ps", bufs=4, space="PSUM") as pspool,
            tc.tile_pool(name="ps2", bufs=2, space="PSUM") as ps2pool,
        ):
            idf2 = cpool.tile([HI, LO], mybir.dt.float32, tag="idf2")
            nc.sync.dma_start(out=idf2[:], in_=idf2_t.ap())
            iota = cpool.tile([128, LO], mybir.dt.float16, tag="iota")
            nc.sync.dma_start(out=iota[:], in_=iota_t.ap())
            onesc = cpool.tile([128, 1], mybir.dt.float32, tag="onesc")
            nc.sync.dma_start(out=onesc[:], in_=onesc_t.ap())
            onesr = cpool.tile([1, 128], mybir.dt.float32, tag="onesr")
            nc.sync.dma_start(out=onesr[:], in_=onesr_t.ap())

            hif = cpool.tile([128, ncols], mybir.dt.float32, tag="hif")
            nc.sync.dma_start(out=hif[:], in_=hif_t.ap())
            lof = cpool.tile([128, ncols], mybir.dt.float32, tag="lof")
            nc.sync.dma_start(out=lof[:], in_=lof_t.ap())

            for g in range(BL // GROUP):
                nsums = wpool.tile([128, GROUP], mybir.dt.float32, tag="nsums")
                Ts = []
                for r in range(GROUP):
                    row = g * GROUP + r
                    C = pspool.tile([HI, LO], mybir.dt.float32, tag="C")
                    for c in range(CH):
                        col = row * CH + c
                        A = wpool.tile([128, HI], mybir.dt.float16, tag="A")
                        nc.vector.tensor_scalar(
                            out=A[:],
                            in0=iota[:, :HI],
                            scalar1=hif[:, col : col + 1],
                            scalar2=None,
                            op0=OP.is_equal,
                        )
                        Bt = wpool.tile([128, LO], mybir.dt.float16, tag="B")
                        nc.vector.tensor_scalar(
                            out=Bt[:],
                            in0=iota[:],
                            scalar1=lof[:, col : col + 1],
                            scalar2=None,
                            op0=OP.is_equal,
                        )
                        nc.tensor.matmul(
                            out=C[:],
                            lhsT=A[:],
                            rhs=Bt[:],
                            start=(c == 0),
                            stop=(c == CH - 1),
                        )
                    T = tpool.tile([HI, LO], mybir.dt.float32, tag="T")
                    nc.vector.tensor_tensor(out=T[:], in0=C[:], in1=idf2[:], op=OP.mult)
                    nc.vector.tensor_reduce(
                        out=nsums[:, r : r + 1],
                        in_=T[:],
                        axis=mybir.AxisListType.X,
                        op=OP.add,
                    )
                    Ts.append(T)
                n_ps = ps2pool.tile([1, GROUP], mybir.dt.float32, tag="nps")
                nc.tensor.matmul(
                    out=n_ps[:], lhsT=onesc[:], rhs=nsums[:], start=True, stop=True
                )
                recip = wpool.tile([1, GROUP], mybir.dt.float32, tag="recip")
                nc.vector.reciprocal(out=recip[:], in_=n_ps[:])
                rb_ps = ps2pool.tile([128, GROUP], mybir.dt.float32, tag="rbps")
                nc.tensor.matmul(
                    out=rb_ps[:], lhsT=onesr[:], rhs=recip[:], start=True, stop=True
                )
                rb = wpool.tile([128, GROUP], mybir.dt.float32, tag="rb")
                nc.vector.tensor_copy(out=rb[:], in_=rb_ps[:])
                for r in range(GROUP):
                    row = g * GROUP + r
                    nc.scalar.activation(
                        out=Ts[r][:],
                        in_=Ts[r][:],
                        func=AF.Copy,
                        scale=rb[:, r : r + 1],
                    )
                    nc.sync.dma_start(out=ov[row], in_=Ts[r][:])
    nc.compile()
    return nc


def _get_nc():
    if "nc" not in _cache:
        _cache["nc"] = _build()
    return _cache["nc"]


def _host_inputs(x: np.ndarray, idf: np.ndarray):
    """Build per-core input maps from the full inputs."""
    idf_pad = np.zeros(VP, dtype=np.float32)
    idf_pad[:V] = np.asarray(idf, dtype=np.float32)
    idf2 = idf_pad.reshape(HI, LO)
    iota = np.broadcast_to(
        np.arange(LO, dtype=np.float16), (128, LO)
    ).copy()
    onesc = np.ones((128, 1), dtype=np.float32)
    onesr = np.ones((1, 128), dtype=np.float32)

    xi = np.asarray(x, dtype=np.int32)  # values < 2**31, safe cast
    hi_all = (xi // LO).astype(np.float32)
    lo_all = (xi % LO).astype(np.float32)
    in_maps = []
    for k in range(NC):
        # layout [128, BL*CH]: element [p, b*CH+c] = v[b, c*128+p]
        def lay(a):
            ac = a[k * BL : (k + 1) * BL]
            return np.ascontiguousarray(
                ac.reshape(BL, CH, 128).transpose(2, 0, 1).reshape(128, BL * CH)
            )
        in_maps.append(
            {
                "hif": lay(hi_all),
                "lof": lay(lo_all),
                "idf2": idf2,
                "iota": iota,
                "onesc": onesc,
                "onesr": onesr,
            }
        )
    return in_maps


def kernel(x: np.ndarray, idf: np.ndarray) -> np.ndarray:
    nc = _get_nc()
    in_maps = _host_inputs(x, idf)
    res = bass_utils.run_bass_kernel_spmd(nc, in_maps, core_ids=list(range(NC)))
    out = np.concatenate([r["out"][:, :V] for r in res.results], axis=0)
    return out


# revision 5
# speedup vs baseline: 1.1432x; 1.1432x over previous
"""TF-IDF document model (histogram_binning) on 8 TRN2 NeuronCores.

Algorithm (per core, 64 batch rows, data-parallel over batch):
  For each row b: tf histogram over vocab V=50257 computed as a radix
  one-hot matmul on the PE: vocab index v = hi*393 + lo with hi in
  [0,128), lo in [0,393). For each 128-token chunk of the row,
  A[s,hi] = (hi_s == hi), B[s,lo] = (lo_s == lo) (fp16 one-hots built
  by DVE tensor_scalar is_equal against an iota tile), and
  C[hi,lo] += A^T @ B accumulates the count matrix in PSUM.
  Then T = C * idf2 (idf reshaped [128,393]) with a fused per-partition
  row-sum; n_b = total sum via a ones-matmul; out_row = T / n_b.
Output written as [64, 50304] per core (vocab padded 50257->50304);
host slices/concats to (512, 50257).
"""
import numpy as np

import concourse.bacc as bacc
import concourse.mybir as mybir
from concourse import bass_utils
from concourse.tile import TileContext

B, S, V = 512, 1024, 50257
NC = 8
BL = B // NC          # 64 rows per core
HI, LO = 128, 393     # radix split: v = hi*LO + lo
VP = HI * LO          # 50304 padded vocab
CH = S // 128         # 8 token chunks per row
GROUP = 8             # rows per normalization group

_cache = {}


def _build():
    nc = bacc.Bacc(
        "TRN2",
        target_bir_lowering=False,
        debug=False,
        enable_asserts=False,
        num_devices=NC,
    )
    hif_t = nc.dram_tensor("hif", [128, BL * CH], mybir.dt.float32, kind="ExternalInput")
    lof_t = nc.dram_tensor("lof", [128, BL * CH], mybir.dt.float32, kind="ExternalInput")
    idf2_t = nc.dram_tensor("idf2", [HI, LO], mybir.dt.float32, kind="ExternalInput")
    iota_t = nc.dram_tensor("iota", [128, LO], mybir.dt.float16, kind="ExternalInput")
    onesc_t = nc.dram_tensor("onesc", [128, 1], mybir.dt.float32, kind="ExternalInput")
    onesr_t = nc.dram_tensor("onesr", [1, 128], mybir.dt.float32, kind="ExternalInput")
    out_t = nc.dram_tensor("out", [BL, VP], mybir.dt.float32, kind="ExternalOutput")
    ov = out_t.ap().rearrange("b (p f) -> b p f", p=HI)
    # group view: [n_groups, p, r, f] matching SBUF staging [128, GROUP*LO]
    ovg = out_t.ap().rearrange("(g r) (p f) -> g p r f", r=GROUP, p=HI)

    AF = mybir.ActivationFunctionType
    OP = mybir.AluOpType
    ncols = BL * CH

    with TileContext(nc) as tc:
        with (
            tc.tile_pool(name="const", bufs=1) as cpool,
            tc.tile_pool(name="work", bufs=4) as wpool,
            tc.tile_pool(name="tt", bufs=3) as tpool,
            tc.tile_pool(name="ps", bufs=4, space="PSUM") as pspool,
            tc.tile_pool(name="ps2", bufs=2, space="PSUM") as ps2pool,
        ):
            idf2 = cpool.tile([HI, LO], mybir.dt.float32, tag="idf2")
            nc.sync.dma_start(out=idf2[:], in_=idf2_t.ap())
            iota = cpool.tile([128, LO], mybir.dt.float16, tag="iota")
            nc.sync.dma_start(out=iota[:], in_=iota_t.ap())
            onesc = cpool.tile([128, 1], mybir.dt.float32, tag="onesc")
            nc.sync.dma_start(out=onesc[:], in_=onesc_t.ap())
            onesr = cpool.tile([1, 128], mybir.dt.float32, tag="onesr")
            nc.sync.dma_start(out=onesr[:], in_=onesr_t.ap())

            hif = cpool.tile([128, ncols], mybir.dt.float32, tag="hif")
            nc.sync.dma_start(out=hif[:], in_=hif_t.ap())
            lof = cpool.tile([128, ncols], mybir.dt.float32, tag="lof")
            nc.sync.dma_start(out=lof[:], in_=lof_t.ap())

            for g in range(BL // GROUP):
                nsums = wpool.tile([128, GROUP], mybir.dt.float32, tag="nsums")
                Tg = tpool.tile([128, GROUP * LO], mybir.dt.float32, tag="Tg")
                for r in range(GROUP):
                    row = g * GROUP + r
                    C = pspool.tile([HI, LO], mybir.dt.float32, tag="C")
                    for c in range(CH):
                        col = row * CH + c
                        A = wpool.tile([128, HI], mybir.dt.float16, tag="A")
                        nc.vector.tensor_scalar(
                            out=A[:],
                            in0=iota[:, :HI],
                            scalar1=hif[:, col : col + 1],
                            scalar2=None,
                            op0=OP.is_equal,
                        )
                        Bt = wpool.tile([128, LO], mybir.dt.float16, tag="B")
                        nc.vector.tensor_scalar(
                            out=Bt[:],
                            in0=iota[:],
                            scalar1=lof[:, col : col + 1],
                            scalar2=None,
                            op0=OP.is_equal,
                        )
                        nc.tensor.matmul(
                            out=C[:],
                            lhsT=A[:],
                            rhs=Bt[:],
                            start=(c == 0),
                            stop=(c == CH - 1),
                        )
                    nc.vector.scalar_tensor_tensor(
                        out=Tg[:, r * LO : (r + 1) * LO],
                        in0=C[:],
                        scalar=1.0,
                        in1=idf2[:],
                        op0=OP.mult,
                        op1=OP.mult,
                        accum_out=nsums[:, r : r + 1],
                    )
                n_ps = ps2pool.tile([1, GROUP], mybir.dt.float32, tag="nps")
                nc.tensor.matmul(
                    out=n_ps[:], lhsT=onesc[:], rhs=nsums[:], start=True, stop=True
                )
                recip = wpool.tile([1, GROUP], mybir.dt.float32, tag="recip")
                nc.vector.reciprocal(out=recip[:], in_=n_ps[:])
                rb_ps = ps2pool.tile([128, GROUP], mybir.dt.float32, tag="rbps")
                nc.tensor.matmul(
                    out=rb_ps[:], lhsT=onesr[:], rhs=recip[:], start=True, stop=True
                )
                rb = wpool.tile([128, GROUP], mybir.dt.float32, tag="rb")
                nc.vector.tensor_copy(out=rb[:], in_=rb_ps[:])
                for r in range(GROUP):
                    nc.scalar.activation(
                        out=Tg[:, r * LO : (r + 1) * LO],
                        in_=Tg[:, r * LO : (r + 1) * LO],
                        func=AF.Copy,
                        scale=rb[:, r : r + 1],
                    )
                nc.sync.dma_start(out=ovg[g], in_=Tg[:])
    nc.compile()
    return nc


def _get_nc():
    if "nc" not in _cache:
        _cache["nc"] = _build()
    return _cache["nc"]


def _host_inputs(x: np.ndarray, idf: np.ndarray):
    """Build per-core input maps from the full inputs."""
    idf_pad = np.zeros(VP, dtype=np.float32)
    idf_pad[:V] = np.asarray(idf, dtype=np.float32)
    idf2 = idf_pad.reshape(HI, LO)
    iota = np.broadcast_to(
        np.arange(LO, dtype=np.float16), (128, LO)
    ).copy()
    onesc = np.ones((128, 1), dtype=np.float32)
    onesr = np.ones((1, 128), dtype=np.float32)

    xi = np.asarray(x, dtype=np.int32)  # values < 2**31, safe cast
    hi_all = (xi // LO).astype(np.float32)
    lo_all = (xi % LO).astype(np.float32)
    in_maps = []
    for k in range(NC):
        # layout [128, BL*CH]: element [p, b*CH+c] = v[b, c*128+p]
        def lay(a):
            ac = a[k * BL : (k + 1) * BL]
            return np.ascontiguousarray(
                ac.reshape(BL, CH, 128).transpose(2, 0, 1).reshape(128, BL * CH)
            )
        in_maps.append(
            {
                "hif": lay(hi_all),
                "lof": lay(lo_all),
                "idf2": idf2,
                "iota": iota,
                "onesc": onesc,
                "onesr": onesr,
            }
        )
    return in_maps


def kernel(x: np.ndarray, idf: np.ndarray) -> np.ndarray:
    nc = _get_nc()
    in_maps = _host_inputs(x, idf)
    res = bass_utils.run_bass_kernel_spmd(nc, in_maps, core_ids=list(range(NC)))
    out = np.concatenate([r["out"][:, :V] for r in res.results], axis=0)
    return out


# revision 6
# speedup vs baseline: 1.3555x; 1.1857x over previous
"""TF-IDF document model (histogram_binning) on 8 TRN2 NeuronCores.

Algorithm (per core, 64 batch rows, data-parallel over batch):
  For each row b: tf histogram over vocab V=50257 computed as a radix
  one-hot matmul on the PE: vocab index v = hi*393 + lo with hi in
  [0,128), lo in [0,393). For each 128-token chunk of the row,
  A[s,hi] = (hi_s == hi), B[s,lo] = (lo_s == lo) (fp16 one-hots built
  by DVE tensor_scalar is_equal against an iota tile), and
  C[hi,lo] += A^T @ B accumulates the count matrix in PSUM.
  Then T = C * idf2 (idf reshaped [128,393]) with a fused per-partition
  row-sum; n_b = total sum via a ones-matmul; out_row = T / n_b.
Output written as [64, 50304] per core (vocab padded 50257->50304);
host slices/concats to (512, 50257).
"""
import numpy as np

import concourse.bacc as bacc
import concourse.mybir as mybir
from concourse import bass_utils
from concourse.tile import TileContext

B, S, V = 512, 1024, 50257
NC = 8
BL = B // NC          # 64 rows per core
HI, LO = 128, 393     # radix split: v = hi*LO + lo
VP = HI * LO          # 50304 padded vocab
CH = S // 128         # 8 token chunks per row
GROUP = 8             # rows per normalization group

_cache = {}


def _build():
    nc = bacc.Bacc(
        "TRN2",
        target_bir_lowering=False,
        debug=False,
        enable_asserts=False,
        num_devices=NC,
    )
    hif_t = nc.dram_tensor("hif", [128, BL * CH], mybir.dt.float32, kind="ExternalInput")
    lof_t = nc.dram_tensor("lof", [128, BL * CH], mybir.dt.float32, kind="ExternalInput")
    loix_t = nc.dram_tensor("loix", [128, 2 * BL * CH], mybir.dt.int16, kind="ExternalInput")
    idf2_t = nc.dram_tensor("idf2", [HI, LO], mybir.dt.float32, kind="ExternalInput")
    iota_t = nc.dram_tensor("iota", [128, LO], mybir.dt.float16, kind="ExternalInput")
    onesc_t = nc.dram_tensor("onesc", [128, 1], mybir.dt.float32, kind="ExternalInput")
    onesr_t = nc.dram_tensor("onesr", [1, 128], mybir.dt.float32, kind="ExternalInput")
    # transposed layout: out[p, b*LO+f] = row b, vocab p*LO+f (host unshuffles)
    out_t = nc.dram_tensor("out", [128, BL * LO], mybir.dt.float32, kind="ExternalOutput")
    ovg = out_t.ap().rearrange("p (g c) -> g p c", g=BL // GROUP)

    AF = mybir.ActivationFunctionType
    OP = mybir.AluOpType
    ncols = BL * CH
    GPB = 3  # B-chunks per row built on GPSIMD local_scatter

    with TileContext(nc) as tc:
        with (
            tc.tile_pool(name="const", bufs=1) as cpool,
            tc.tile_pool(name="work", bufs=4) as wpool,
            tc.tile_pool(name="tt", bufs=3) as tpool,
            tc.tile_pool(name="ps", bufs=4, space="PSUM") as pspool,
            tc.tile_pool(name="ps2", bufs=2, space="PSUM") as ps2pool,
        ):
            idf2 = cpool.tile([HI, LO], mybir.dt.float32, tag="idf2")
            nc.sync.dma_start(out=idf2[:], in_=idf2_t.ap())
            iota = cpool.tile([128, LO], mybir.dt.float16, tag="iota")
            nc.sync.dma_start(out=iota[:], in_=iota_t.ap())
            onesc = cpool.tile([128, 1], mybir.dt.float32, tag="onesc")
            nc.sync.dma_start(out=onesc[:], in_=onesc_t.ap())
            onesr = cpool.tile([1, 128], mybir.dt.float32, tag="onesr")
            nc.sync.dma_start(out=onesr[:], in_=onesr_t.ap())

            hif = cpool.tile([128, ncols], mybir.dt.float32, tag="hif")
            nc.sync.dma_start(out=hif[:], in_=hif_t.ap())
            lof = cpool.tile([128, ncols], mybir.dt.float32, tag="lof")
            nc.sync.dma_start(out=lof[:], in_=lof_t.ap())
            loix = cpool.tile([128, 2 * ncols], mybir.dt.int16, tag="loix")
            nc.sync.dma_start(out=loix[:], in_=loix_t.ap())
            ones16 = cpool.tile([128, 2], mybir.dt.float16, tag="ones16")
            nc.vector.memset(ones16[:], 1.0)

            for g in range(BL // GROUP):
                nsums = wpool.tile([128, GROUP], mybir.dt.float32, tag="nsums")
                Tg = tpool.tile([128, GROUP * LO], mybir.dt.float32, tag="Tg")
                for r in range(GROUP):
                    row = g * GROUP + r
                    C = pspool.tile([HI, LO], mybir.dt.float32, tag="C")
                    for c in range(CH):
                        col = row * CH + c
                        A = wpool.tile([128, HI], mybir.dt.float16, tag="A")
                        nc.vector.tensor_scalar(
                            out=A[:],
                            in0=iota[:, :HI],
                            scalar1=hif[:, col : col + 1],
                            scalar2=None,
                            op0=OP.is_equal,
                        )
                        if c < GPB:
                            Bg = wpool.tile([128, 394], mybir.dt.float16, tag="Bg")
                            nc.gpsimd.local_scatter(
                                out_ap=Bg[:],
                                data_ap=ones16[:],
                                idxs_ap=loix[:, 2 * col : 2 * col + 2],
                                channels=128,
                                num_elems=394,
                                num_idxs=2,
                            )
                            rhs = Bg[:, :LO]
                        else:
                            Bt = wpool.tile([128, LO], mybir.dt.float16, tag="B")
                            nc.vector.tensor_scalar(
                                out=Bt[:],
                                in0=iota[:],
                                scalar1=lof[:, col : col + 1],
                                scalar2=None,
                                op0=OP.is_equal,
                            )
                            rhs = Bt[:]
                        nc.tensor.matmul(
                            out=C[:],
                            lhsT=A[:],
                            rhs=rhs,
                            start=(c == 0),
                            stop=(c == CH - 1),
                        )
                    nc.vector.scalar_tensor_tensor(
                        out=Tg[:, r * LO : (r + 1) * LO],
                        in0=C[:],
                        scalar=1.0,
                        in1=idf2[:],
                        op0=OP.mult,
                        op1=OP.mult,
                        accum_out=nsums[:, r : r + 1],
                    )
                n_ps = ps2pool.tile([1, GROUP], mybir.dt.float32, tag="nps")
                nc.tensor.matmul(
                    out=n_ps[:], lhsT=onesc[:], rhs=nsums[:], start=True, stop=True
                )
                recip = wpool.tile([1, GROUP], mybir.dt.float32, tag="recip")
                nc.vector.reciprocal(out=recip[:], in_=n_ps[:])
                rb_ps = ps2pool.tile([128, GROUP], mybir.dt.float32, tag="rbps")
                nc.tensor.matmul(
                    out=rb_ps[:], lhsT=onesr[:], rhs=recip[:], start=True, stop=True
                )
                rb = wpool.tile([128, GROUP], mybir.dt.float32, tag="rb")
                nc.vector.tensor_copy(out=rb[:], in_=rb_ps[:])
                for r in range(GROUP):
                    nc.scalar.activation(
                        out=Tg[:, r * LO : (r + 1) * LO],
                        in_=Tg[:, r * LO : (r + 1) * LO],
                        func=AF.Copy,
                        scale=rb[:, r : r + 1],
                    )
                nc.sync.dma_start(out=ovg[g], in_=Tg[:])
    nc.compile()
    return nc


def _get_nc():
    if "nc" not in _cache:
        _cache["nc"] = _build()
    return _cache["nc"]


def _host_inputs(x: np.ndarray, idf: np.ndarray):
    """Build per-core input maps from the full inputs."""
    idf_pad = np.zeros(VP, dtype=np.float32)
    idf_pad[:V] = np.asarray(idf, dtype=np.float32)
    idf2 = idf_pad.reshape(HI, LO)
    iota = np.broadcast_to(
        np.arange(LO, dtype=np.float16), (128, LO)
    ).copy()
    onesc = np.ones((128, 1), dtype=np.float32)
    onesr = np.ones((1, 128), dtype=np.float32)

    xi = np.asarray(x, dtype=np.int32)  # values < 2**31, safe cast
    hi_all = (xi // LO).astype(np.float32)
    lo_all = (xi % LO).astype(np.float32)
    in_maps = []
    for k in range(NC):
        # layout [128, BL*CH]: element [p, b*CH+c] = v[b, c*128+p]
        def lay(a):
            ac = a[k * BL : (k + 1) * BL]
            return np.ascontiguousarray(
                ac.reshape(BL, CH, 128).transpose(2, 0, 1).reshape(128, BL * CH)
            )
        lo_l = lay(lo_all).astype(np.int16)          # [128, ncols]
        loix = np.empty((128, 2 * lo_l.shape[1]), np.int16)
        loix[:, 0::2] = lo_l
        loix[:, 1::2] = 393
        in_maps.append(
            {
                "hif": lay(hi_all),
                "lof": lay(lo_all),
                "loix": loix,
                "idf2": idf2,
                "iota": iota,
                "onesc": onesc,
                "onesr": onesr,
            }
        )
    return in_maps


def kernel(x: np.ndarray, idf: np.ndarray) -> np.ndarray:
    nc = _get_nc()
    in_maps = _host_inputs(x, idf)
    res = bass_utils.run_bass_kernel_spmd(nc, in_maps, core_ids=list(range(NC)))
    outs = []
    for r in res.results:
        a = r["out"].reshape(128, BL, LO).transpose(1, 0, 2).reshape(BL, VP)
        outs.append(a[:, :V])
    return np.concatenate(outs, axis=0)
